# revision 14
# baseline (speedup 1.0000x reference)
"""Cross-modal attention TRN2 kernel (v2, bf16).

Problem: B=4, N=2048, IN_DIM=DIM=1024, HEADS=8, D_HEAD=128, scale=DIM**-0.5.
  q = x_a @ W_q.T ; k,v = split(x_b @ W_kv.T) ; per-head softmax(q k^T/32) v ;
  out = merge_heads @ W_out.T + b_out

Sharding over 8 cores: core c -> batch b=c//2, head-half hh=c%2 (4 heads,
512 of DIM).  W_q/W_kv column-sharded, W_out row-sharded (Megatron); each
core emits a partial output projection y_cT = (W_out[:, slice] @ O_half)
of shape [DIM, N] in bf16; host sums the two head-half partials per batch
in fp32, adds b_out, transposes back.

v2 changes vs v1 (fp32r, 401us):
  - all matmul operands bf16 (same 1 cyc/row PE rate, but half DMA, half
    LDWEIGHTS, FWL enabled, 2x DVE rates).  PSUM accumulation stays fp32.
  - softmax denominator no longer computed with per-j-tile ones-matmuls
    (which cost as much PE time as the PV matmuls).  Instead the exp tiles
    are summed over j-tiles with a bf16 binary add-tree on the Vector
    engine, and one [128,128] all-ones stationary matmul per (ib,h) both
    reduces over the 128 j-partitions and broadcasts the denominator to
    all 128 partitions of a PSUM tile.  reciprocal_approx_fast (DVE custom
    op, ~5x faster than InstReciprocal) gives 1/den at [128,1024] shape,
    so nothing runs at 1-partition serial rates and the per-iteration
    normalize chain is off the PE critical path.
  - phase 2 software-pipelined ACROSS (ib,h) iterations (PE never drains,
    so the HAM clock gate stays at 2.4 GHz), with the phase-3 output
    projection matmuls for i-block 0 interleaved into the PE slack of the
    ACT-bound (exp-bound) attention iterations of i-block 1.
"""

import numpy as np
from collections import deque

B, N, IN_DIM, DIM, HEADS = 4, 2048, 1024, 1024, 8
D_HEAD = DIM // HEADS          # 128
SCALE = DIM ** -0.5            # 1/32
NCORES = 8
HH = HEADS // 2                # 4 heads per core
DVC = HH * D_HEAD              # 512 dv per core
P = 128
KT = IN_DIM // P               # 8 contraction tiles
NJT = N // P                   # 16 j tiles
IB2 = N // 1024                # 2 i-blocks of 1024
LAG = 2                        # PV trails dots/exp by LAG j-tiles

_TRACE = False
_TRACE_DIR = None
REPS = 1
LAST_EXEC_NS = None
LAST_RESULTS = None


def _build_nc(reps=1):
    import concourse.tile as tile
    from concourse import bacc, mybir

    f32 = mybir.dt.float32
    bf16 = mybir.dt.bfloat16
    Exp = mybir.ActivationFunctionType.Exp

    nc = bacc.Bacc("TRN2", debug=False, num_devices=NCORES)

    # Everything bf16 (fp8e4 DoubleRow q/k projections were measured at
    # rel_err 1.96e-2 vs the 2e-2 gate -- too thin; bf16 runs at 6.5e-3).
    xaT = nc.dram_tensor("xaT", [IN_DIM, N], bf16, kind="ExternalInput").ap()
    xbT = nc.dram_tensor("xbT", [IN_DIM, N], bf16, kind="ExternalInput").ap()
    wqT = nc.dram_tensor("wqT", [IN_DIM, DVC], bf16, kind="ExternalInput").ap()
    wkT = nc.dram_tensor("wkT", [IN_DIM, DVC], bf16, kind="ExternalInput").ap()
    wvT = nc.dram_tensor("wvT", [IN_DIM, DVC], bf16, kind="ExternalInput").ap()
    woT = nc.dram_tensor("woT", [DVC, DIM], bf16, kind="ExternalInput").ap()
    ones_d = nc.dram_tensor("ones", [P, P], bf16, kind="ExternalInput").ap()
    yT = nc.dram_tensor("yT", [DIM, N], bf16, kind="ExternalOutput").ap()

    with tile.TileContext(nc) as tc:
      for _rep in range(reps):
        with tc.tile_pool(name="persist", bufs=1) as persist:
            qT_sb = persist.tile([P, HH, N], bf16, tag="qT")    # [d%128, h, i]
            kT_sb = persist.tile([P, HH, N], bf16, tag="kT")    # [d%128, h, j]
            v_sb = persist.tile([P, NJT, DVC], bf16, tag="v")   # [j%128, jt, dv]
            o_ts = [[persist.tile([P, 1024], bf16, tag=f"o{h}_{bb}",
                                  name=f"o{h}_{bb}")
                     for bb in range(IB2)] for h in range(HH)]
            ones_sb = persist.tile([P, P], bf16, tag="ones")
            wo_sb = persist.tile([P, HH, DIM], bf16, tag="wo")  # [dv%128, h, e]

            # ---------------- phase 1: projections ----------------
            BW = 512
            NB = N // BW                                        # 4 blocks
            with tc.tile_pool(name="wpool", bufs=1) as wpool, \
                 tc.tile_pool(name="xapool", bufs=2) as xapool, \
                 tc.tile_pool(name="xbpool", bufs=2) as xbpool, \
                 tc.tile_pool(name="psum1", bufs=4, space="PSUM") as psum1:
                wq_sb = wpool.tile([P, KT, DVC], bf16, tag="wq")
                wk_sb = wpool.tile([P, KT, DVC], bf16, tag="wk")
                wv_sb = wpool.tile([P, KT, DVC], bf16, tag="wv")

                def new_xa(blk):
                    t = xapool.tile([P, KT, BW], bf16, tag="xa", name="xa_blk")
                    nc.sync.dma_start(
                        out=t,
                        in_=xaT[:, blk * BW:(blk + 1) * BW]
                        .rearrange("(kt p) i -> p kt i", p=P))
                    return t

                def new_xb(blk):
                    t = xbpool.tile([P, KT, BW], bf16, tag="xb", name="xb_blk")
                    nc.sync.dma_start(
                        out=t,
                        in_=xbT[:, blk * BW:(blk + 1) * BW]
                        .rearrange("(kt p) i -> p kt i", p=P))
                    return t

                # DMA order: earliest-needed first
                xa_blk = new_xa(0)
                nc.sync.dma_start(
                    out=wq_sb, in_=wqT.rearrange("(kt p) d -> p kt d", p=P))
                xb_blk = new_xb(0)
                nc.sync.dma_start(
                    out=wk_sb, in_=wkT.rearrange("(kt p) d -> p kt d", p=P))
                nc.sync.dma_start(
                    out=wv_sb, in_=wvT.rearrange("(kt p) d -> p kt d", p=P))
                nc.sync.dma_start(out=ones_sb, in_=ones_d)
                nc.sync.dma_start(
                    out=wo_sb, in_=woT.rearrange("(dt p) e -> p dt e", p=P))

                for blk in range(NB):
                    if blk > 0:
                        xa_blk = new_xa(blk)
                        xb_blk = new_xb(blk)
                    # Q block
                    for dt in range(HH):
                        ps = psum1.tile([P, BW], f32, tag="ps1", name="ps1")
                        for kt in range(KT):
                            nc.tensor.matmul(
                                ps, wq_sb[:, kt, dt * P:(dt + 1) * P],
                                xa_blk[:, kt, :],
                                start=(kt == 0), stop=(kt == KT - 1))
                        nc.vector.tensor_copy(
                            qT_sb[:, dt, blk * BW:(blk + 1) * BW], ps)
                    # K block
                    for dt in range(HH):
                        ps = psum1.tile([P, BW], f32, tag="ps1", name="ps1")
                        for kt in range(KT):
                            nc.tensor.matmul(
                                ps, wk_sb[:, kt, dt * P:(dt + 1) * P],
                                xb_blk[:, kt, :],
                                start=(kt == 0), stop=(kt == KT - 1))
                        nc.vector.tensor_copy(
                            kT_sb[:, dt, blk * BW:(blk + 1) * BW], ps)
                    # V block (j-partitioned: stationary = x slice)
                    for j2 in range(BW // P):
                        ps = psum1.tile([P, DVC], f32, tag="ps1", name="psv")
                        for kt in range(KT):
                            nc.tensor.matmul(
                                ps, xb_blk[:, kt, j2 * P:(j2 + 1) * P],
                                wv_sb[:, kt, :],
                                start=(kt == 0), stop=(kt == KT - 1))
                        nc.vector.tensor_copy(
                            v_sb[:, blk * (BW // P) + j2, :], ps)

            # ---------------- phase 2 + 3: attention + out-proj ----------
            with tc.tile_pool(name="expp", bufs=6) as expp, \
                 tc.tile_pool(name="treep", bufs=6) as treep, \
                 tc.tile_pool(name="rcp", bufs=2) as rcp, \
                 tc.tile_pool(name="ysp", bufs=4) as ysp, \
                 tc.tile_pool(name="dotsp", bufs=2, space="PSUM") as dotsp, \
                 tc.tile_pool(name="avp", bufs=1, space="PSUM") as avp, \
                 tc.tile_pool(name="psyp", bufs=1, space="PSUM") as psyp:

                iters = [(ib, h) for ib in range(IB2) for h in range(HH)]
                pv_pend = deque()      # (k, jd, et)
                ph3_queue = deque()    # (ib, e8, hf)
                state = {}             # k -> dict(acc, pd, rc, po)

                def feed_tree(levels, cur):
                    lvl = 0
                    while levels[lvl] is not None:
                        prev = levels[lvl]
                        levels[lvl] = None
                        with nc.allow_low_precision("softmax denom tree bf16"):
                            dst = treep.tile([P, 1024], bf16, tag="tree",
                                             name="tree")
                            nc.vector.tensor_add(dst, prev, cur)
                        cur = dst
                        lvl += 1
                    levels[lvl] = cur

                def emit_pv(kk, jd, et):
                    st = state[kk]
                    _, hh_ = iters[kk]
                    v_l = v_sb[:, jd, hh_ * P:(hh_ + 1) * P]
                    for hf in range(2):
                        sl = slice(hf * 512, (hf + 1) * 512)
                        nc.tensor.matmul(
                            st["po"][:, sl], v_l, et[:, sl],
                            start=(jd == 0), stop=(jd == NJT - 1))
                    if jd == NJT - 1:
                        # drain PV accumulator right away so the single
                        # avp buffer frees for the next iteration
                        ib_, hh2 = iters[kk]
                        nc.vector.tensor_copy(o_ts[hh2][ib_], st["po"])

                def emit_den(kk):
                    st = state[kk]
                    pd = psyp.tile([P, 1024], f32, tag="psy", name="pden")
                    for hf in range(2):
                        sl = slice(hf * 512, (hf + 1) * 512)
                        nc.tensor.matmul(pd[:, sl], ones_sb, st["acc"][:, sl],
                                         start=True, stop=True)
                    st["pd"] = pd

                def emit_recip(kk):
                    st = state[kk]
                    rc = rcp.tile([P, 1024], f32, tag="rc", name="rc")
                    nc.vector.reciprocal_approx_fast(rc, st["pd"])
                    st["rc"] = rc

                def emit_norm(kk):
                    st = state[kk]
                    ib_, hh_ = iters[kk]
                    osl = o_ts[hh_][ib_]
                    with nc.allow_low_precision("softmax normalize bf16"):
                        nc.vector.tensor_mul(osl, osl, st["rc"])

                def emit_ph3_chunk(ib_, e8, hf, copy_eng="v"):
                    pt = psyp.tile([P, 1024], f32, tag="psy", name="py")
                    pz = pt[:, 0:512]
                    for dt in range(HH):
                        nc.tensor.matmul(
                            pz, wo_sb[:, dt, e8 * P:(e8 + 1) * P],
                            o_ts[dt][ib_][:, hf * 512:(hf + 1) * 512],
                            start=(dt == 0), stop=(dt == HH - 1))
                    ys = ysp.tile([P, 512], bf16, tag="ys", name="ys")
                    if copy_eng == "v":
                        nc.vector.tensor_copy(ys, pz)
                    else:
                        nc.scalar.copy(ys, pz)
                    i0 = ib_ * 1024 + hf * 512
                    nc.sync.dma_start(
                        out=yT[e8 * P:(e8 + 1) * P, i0:i0 + 512], in_=ys)

                for k, (ib, h) in enumerate(iters):
                    st = {"po": avp.tile([P, 1024], f32, tag="po", name="po")}
                    state[k] = st
                    levels = [None] * 5
                    i_base = ib * 1024

                    for jt in range(NJT):
                        # deferred post-iteration work for k-1, spread over
                        # early steps so nothing serializes the PE
                        if k > 0:
                            if jt == 2:
                                emit_den(k - 1)
                            elif jt == 3:
                                emit_recip(k - 1)
                            elif jt == 5:
                                emit_norm(k - 1)

                        # dots
                        ps = dotsp.tile([P, 1024], f32, tag="ps", name="ps")
                        k_l = kT_sb[:, h, jt * P:(jt + 1) * P]
                        for hf in range(2):
                            nc.tensor.matmul(
                                ps[:, hf * 512:(hf + 1) * 512], k_l,
                                qT_sb[:, h,
                                      i_base + hf * 512:i_base + (hf + 1) * 512],
                                start=True, stop=True)
                        et = expp.tile([P, 1024], bf16, tag="exp", name="et")
                        nc.scalar.activation(et, ps, Exp, scale=SCALE)
                        feed_tree(levels, et)
                        pv_pend.append((k, jt, et))
                        if len(pv_pend) > LAG:
                            emit_pv(*pv_pend.popleft())

                        # interleave phase-3 chunks of the previous i-block
                        # into the PE slack (ACT-bound steps); jt>=5 keeps
                        # the psy slot free for den/recip of k-1 and (at
                        # k==4) waits for osl(ib0,h3) to be normalized
                        if ph3_queue and jt in (5, 8, 11, 14):
                            emit_ph3_chunk(*ph3_queue.popleft())

                    st["acc"] = levels[4]
                    assert st["acc"] is not None

                    if h == HH - 1:
                        for e8 in range(DIM // P):
                            for hf in range(2):
                                ph3_queue.append((ib, e8, hf))

                # tail: drain the pipeline for the last iteration.  The ib1
                # out-proj chunks run 2-deep through the (now free) dots
                # PSUM pool, with their dt<3 accumulations pre-running while
                # the final denominator chain completes on DVE, so the PE
                # never idles long enough for HAM to re-throttle the clock.
                while pv_pend:
                    emit_pv(*pv_pend.popleft())
                kl = len(iters) - 1
                tail = list(ph3_queue)
                ph3_queue.clear()
                open_ps = {}
                LOOK = 3   # 3 open out-proj groups: 2 dotsp bufs + the avp
                           # buffer (free once po[kl] is drained)

                def tail_stage_a(i):
                    ib_, e8, hf = tail[i]
                    if i % 3 == 2:
                        pt = avp.tile([P, 1024], f32, tag="po", name="po_t")
                    else:
                        pt = dotsp.tile([P, 1024], f32, tag="ps", name="py2")
                    pz = pt[:, 0:512]
                    for dt in range(HH - 1):
                        nc.tensor.matmul(
                            pz, wo_sb[:, dt, e8 * P:(e8 + 1) * P],
                            o_ts[dt][ib_][:, hf * 512:(hf + 1) * 512],
                            start=(dt == 0), stop=False)
                    open_ps[i] = pt

                def tail_stage_b(j):
                    ib_, e8, hf = tail[j]
                    pt = open_ps.pop(j)
                    pz = pt[:, 0:512]
                    nc.tensor.matmul(
                        pz, wo_sb[:, HH - 1, e8 * P:(e8 + 1) * P],
                        o_ts[HH - 1][ib_][:, hf * 512:(hf + 1) * 512],
                        start=False, stop=True)
                    # ACT does all tail copies: it is idle here, while DVE
                    # still owes the final reciprocal/normalize chain
                    ys = ysp.tile([P, 512], bf16, tag="ys", name="ys")
                    nc.scalar.copy(ys, pz)
                    i0 = ib_ * 1024 + hf * 512
                    nc.sync.dma_start(
                        out=yT[e8 * P:(e8 + 1) * P, i0:i0 + 512], in_=ys)

                for i in range(len(tail) + LOOK):
                    if i >= LOOK:
                        tail_stage_b(i - LOOK)
                    if i < len(tail):
                        tail_stage_a(i)
                    if i == 0:
                        emit_den(kl)
                        emit_recip(kl)
                    elif i == 1:
                        emit_norm(kl)

    nc.compile()
    return nc


_nc_by_reps = {}


def _get_nc(reps=1):
    if reps not in _nc_by_reps:
        _nc_by_reps[reps] = _build_nc(reps)
    return _nc_by_reps[reps]


def _make_in_maps(x_a, x_b, W_q, W_kv, W_out):
    import ml_dtypes
    bf = ml_dtypes.bfloat16
    xaT = [np.ascontiguousarray(x_a[b].T).astype(bf) for b in range(B)]
    xbT = [np.ascontiguousarray(x_b[b].T).astype(bf) for b in range(B)]
    ones = np.ones((P, P), dtype=bf)
    in_maps = []
    for c in range(NCORES):
        b, hh = divmod(c, 2)
        hs = hh * DVC
        in_maps.append({
            "xaT": xaT[b],
            "xbT": xbT[b],
            "wqT": np.ascontiguousarray(W_q[hs:hs + DVC].T).astype(bf),
            "wkT": np.ascontiguousarray(W_kv[hs:hs + DVC].T).astype(bf),
            "wvT": np.ascontiguousarray(
                W_kv[DIM + hs:DIM + hs + DVC].T).astype(bf),
            "woT": np.ascontiguousarray(W_out[:, hs:hs + DVC].T).astype(bf),
            "ones": ones,
        })
    return in_maps


def kernel(x_a, x_b, W_q, W_kv, W_out, b_out):
    global LAST_EXEC_NS, LAST_RESULTS
    from concourse import bass_utils

    x_a = np.asarray(x_a, dtype=np.float32)
    x_b = np.asarray(x_b, dtype=np.float32)
    W_q = np.asarray(W_q, dtype=np.float32)
    W_kv = np.asarray(W_kv, dtype=np.float32)
    W_out = np.asarray(W_out, dtype=np.float32)
    b_out = np.asarray(b_out, dtype=np.float32)

    nc = _get_nc(REPS)
    in_maps = _make_in_maps(x_a, x_b, W_q, W_kv, W_out)

    res = bass_utils.run_bass_kernel_spmd(
        nc, in_maps, core_ids=list(range(NCORES)), trace=_TRACE,
        tmpdir=_TRACE_DIR)
    LAST_EXEC_NS = res.exec_time_ns
    LAST_RESULTS = res

    out = np.empty((B, N, DIM), dtype=np.float32)
    for b in range(B):
        acc = (np.asarray(res.results[2 * b]["yT"]).astype(np.float32)
               + np.asarray(res.results[2 * b + 1]["yT"]).astype(np.float32))
        out[b] = acc.T + b_out
    return out


def bench(inputs, reps_pair=(1, 9), iters=5):
    """Measure on-device time per kernel body via rep-delta wall timing."""
    import time
    from concourse import bass_utils
    ins = {k: np.asarray(v, dtype=np.float32) for k, v in inputs.items()
           if k != "b_out"}
    in_maps = _make_in_maps(ins["x_a"], ins["x_b"], ins["W_q"], ins["W_kv"],
                            ins["W_out"])
    walls = {}
    for reps in reps_pair:
        nc = _get_nc(reps)
        bass_utils.run_bass_kernel_spmd(nc, in_maps, core_ids=list(range(NCORES)))
        ts = []
        for _ in range(iters):
            t0 = time.perf_counter()
            bass_utils.run_bass_kernel_spmd(nc, in_maps,
                                            core_ids=list(range(NCORES)))
            ts.append(time.perf_counter() - t0)
        walls[reps] = min(ts)
        print(f"reps={reps}: wall min={walls[reps]*1e3:.2f} ms  "
              f"all={[f'{t*1e3:.1f}' for t in ts]}")
    r0, r1 = reps_pair
    ns = (walls[r1] - walls[r0]) / (r1 - r0) * 1e9
    print(f"per-body device time: {ns:.0f} ns")
    return ns


# revision 16
# speedup vs baseline: 1.0047x; 1.0047x over previous
"""Cross-modal attention TRN2 kernel (v2, bf16).

Problem: B=4, N=2048, IN_DIM=DIM=1024, HEADS=8, D_HEAD=128, scale=DIM**-0.5.
  q = x_a @ W_q.T ; k,v = split(x_b @ W_kv.T) ; per-head softmax(q k^T/32) v ;
  out = merge_heads @ W_out.T + b_out

Sharding over 8 cores: core c -> batch b=c//2, head-half hh=c%2 (4 heads,
512 of DIM).  W_q/W_kv column-sharded, W_out row-sharded (Megatron); each
core emits a partial output projection y_cT = (W_out[:, slice] @ O_half)
of shape [DIM, N] in bf16; host sums the two head-half partials per batch
in fp32, adds b_out, transposes back.

v2 changes vs v1 (fp32r, 401us):
  - all matmul operands bf16 (same 1 cyc/row PE rate, but half DMA, half
    LDWEIGHTS, FWL enabled, 2x DVE rates).  PSUM accumulation stays fp32.
  - softmax denominator no longer computed with per-j-tile ones-matmuls
    (which cost as much PE time as the PV matmuls).  Instead the exp tiles
    are summed over j-tiles with a bf16 binary add-tree on the Vector
    engine, and one [128,128] all-ones stationary matmul per (ib,h) both
    reduces over the 128 j-partitions and broadcasts the denominator to
    all 128 partitions of a PSUM tile.  reciprocal_approx_fast (DVE custom
    op, ~5x faster than InstReciprocal) gives 1/den at [128,1024] shape,
    so nothing runs at 1-partition serial rates and the per-iteration
    normalize chain is off the PE critical path.
  - phase 2 software-pipelined ACROSS (ib,h) iterations (PE never drains,
    so the HAM clock gate stays at 2.4 GHz), with the phase-3 output
    projection matmuls for i-block 0 interleaved into the PE slack of the
    ACT-bound (exp-bound) attention iterations of i-block 1.
"""

import numpy as np
from collections import deque

B, N, IN_DIM, DIM, HEADS = 4, 2048, 1024, 1024, 8
D_HEAD = DIM // HEADS          # 128
SCALE = DIM ** -0.5            # 1/32
NCORES = 8
HH = HEADS // 2                # 4 heads per core
DVC = HH * D_HEAD              # 512 dv per core
P = 128
KT = IN_DIM // P               # 8 contraction tiles
NJT = N // P                   # 16 j tiles
IB2 = N // 1024                # 2 i-blocks of 1024
LAG = 2                        # PV trails dots/exp by LAG j-tiles

_TRACE = False
_TRACE_DIR = None
REPS = 1
LAST_EXEC_NS = None
LAST_RESULTS = None


def _build_nc(reps=1):
    import concourse.tile as tile
    from concourse import bacc, mybir

    f32 = mybir.dt.float32
    bf16 = mybir.dt.bfloat16
    Exp = mybir.ActivationFunctionType.Exp

    nc = bacc.Bacc("TRN2", debug=False, num_devices=NCORES)

    # Everything bf16 (fp8e4 DoubleRow q/k projections were measured at
    # rel_err 1.96e-2 vs the 2e-2 gate -- too thin; bf16 runs at 6.5e-3).
    xaT = nc.dram_tensor("xaT", [IN_DIM, N], bf16, kind="ExternalInput").ap()
    xbT = nc.dram_tensor("xbT", [IN_DIM, N], bf16, kind="ExternalInput").ap()
    wqT = nc.dram_tensor("wqT", [IN_DIM, DVC], bf16, kind="ExternalInput").ap()
    wkT = nc.dram_tensor("wkT", [IN_DIM, DVC], bf16, kind="ExternalInput").ap()
    wvT = nc.dram_tensor("wvT", [IN_DIM, DVC], bf16, kind="ExternalInput").ap()
    woT = nc.dram_tensor("woT", [DVC, DIM], bf16, kind="ExternalInput").ap()
    ones_d = nc.dram_tensor("ones", [P, P], bf16, kind="ExternalInput").ap()
    yT = nc.dram_tensor("yT", [DIM, N], bf16, kind="ExternalOutput").ap()

    with tile.TileContext(nc) as tc:
      for _rep in range(reps):
        with tc.tile_pool(name="persist", bufs=1) as persist:
            qT_sb = persist.tile([P, HH, N], bf16, tag="qT")    # [d%128, h, i]
            kT_sb = persist.tile([P, HH, N], bf16, tag="kT")    # [d%128, h, j]
            v_sb = persist.tile([P, NJT, DVC], bf16, tag="v")   # [j%128, jt, dv]
            o_ts = [[persist.tile([P, 1024], bf16, tag=f"o{h}_{bb}",
                                  name=f"o{h}_{bb}")
                     for bb in range(IB2)] for h in range(HH)]
            ones_sb = persist.tile([P, P], bf16, tag="ones")
            wo_sb = persist.tile([P, HH, DIM], bf16, tag="wo")  # [dv%128, h, e]

            # ---------------- phase 1: projections ----------------
            BW = 512
            NB = N // BW                                        # 4 blocks
            with tc.tile_pool(name="wpool", bufs=1) as wpool, \
                 tc.tile_pool(name="xapool", bufs=2) as xapool, \
                 tc.tile_pool(name="xbpool", bufs=2) as xbpool, \
                 tc.tile_pool(name="psum1", bufs=4, space="PSUM") as psum1:
                wq_sb = wpool.tile([P, KT, DVC], bf16, tag="wq")
                wk_sb = wpool.tile([P, KT, DVC], bf16, tag="wk")
                wv_sb = wpool.tile([P, KT, DVC], bf16, tag="wv")

                def new_xa(blk, halves=1):
                    t = xapool.tile([P, KT, BW], bf16, tag="xa", name="xa_blk")
                    hk = KT // halves
                    for hv in range(halves):
                        nc.sync.dma_start(
                            out=t[:, hv * hk:(hv + 1) * hk, :],
                            in_=xaT[hv * hk * P:(hv + 1) * hk * P,
                                    blk * BW:(blk + 1) * BW]
                            .rearrange("(kt p) i -> p kt i", p=P))
                    return t

                def new_xb(blk, halves=1):
                    t = xbpool.tile([P, KT, BW], bf16, tag="xb", name="xb_blk")
                    hk = KT // halves
                    for hv in range(halves):
                        nc.sync.dma_start(
                            out=t[:, hv * hk:(hv + 1) * hk, :],
                            in_=xbT[hv * hk * P:(hv + 1) * hk * P,
                                    blk * BW:(blk + 1) * BW]
                            .rearrange("(kt p) i -> p kt i", p=P))
                    return t

                # DMA order: earliest-needed first; block 0 and the weights
                # are split in kt-halves interleaved so the first Q matmul
                # can issue ~3us in instead of waiting out both full loads.
                def dma_w_half(dst, src, hv):
                    nc.sync.dma_start(
                        out=dst[:, hv * 4:(hv + 1) * 4, :],
                        in_=src[hv * 4 * P:(hv + 1) * 4 * P, :]
                        .rearrange("(kt p) d -> p kt d", p=P))

                xa_blk = xapool.tile([P, KT, BW], bf16, tag="xa",
                                     name="xa_blk")
                nc.sync.dma_start(
                    out=xa_blk[:, 0:4, :],
                    in_=xaT[0:4 * P, 0:BW].rearrange("(kt p) i -> p kt i",
                                                     p=P))
                dma_w_half(wq_sb, wqT, 0)
                nc.sync.dma_start(
                    out=xa_blk[:, 4:8, :],
                    in_=xaT[4 * P:8 * P, 0:BW].rearrange("(kt p) i -> p kt i",
                                                         p=P))
                dma_w_half(wq_sb, wqT, 1)
                xb_blk = new_xb(0, halves=2)
                dma_w_half(wk_sb, wkT, 0)
                dma_w_half(wk_sb, wkT, 1)
                nc.sync.dma_start(
                    out=wv_sb, in_=wvT.rearrange("(kt p) d -> p kt d", p=P))
                nc.sync.dma_start(out=ones_sb, in_=ones_d)
                nc.sync.dma_start(
                    out=wo_sb, in_=woT.rearrange("(dt p) e -> p dt e", p=P))

                for blk in range(NB):
                    if blk > 0:
                        xa_blk = new_xa(blk)
                        xb_blk = new_xb(blk)
                    # Q block
                    for dt in range(HH):
                        ps = psum1.tile([P, BW], f32, tag="ps1", name="ps1")
                        for kt in range(KT):
                            nc.tensor.matmul(
                                ps, wq_sb[:, kt, dt * P:(dt + 1) * P],
                                xa_blk[:, kt, :],
                                start=(kt == 0), stop=(kt == KT - 1))
                        nc.vector.tensor_copy(
                            qT_sb[:, dt, blk * BW:(blk + 1) * BW], ps)
                    # K block
                    for dt in range(HH):
                        ps = psum1.tile([P, BW], f32, tag="ps1", name="ps1")
                        for kt in range(KT):
                            nc.tensor.matmul(
                                ps, wk_sb[:, kt, dt * P:(dt + 1) * P],
                                xb_blk[:, kt, :],
                                start=(kt == 0), stop=(kt == KT - 1))
                        nc.vector.tensor_copy(
                            kT_sb[:, dt, blk * BW:(blk + 1) * BW], ps)
                    # V block (j-partitioned: stationary = x slice)
                    for j2 in range(BW // P):
                        ps = psum1.tile([P, DVC], f32, tag="ps1", name="psv")
                        for kt in range(KT):
                            nc.tensor.matmul(
                                ps, xb_blk[:, kt, j2 * P:(j2 + 1) * P],
                                wv_sb[:, kt, :],
                                start=(kt == 0), stop=(kt == KT - 1))
                        nc.vector.tensor_copy(
                            v_sb[:, blk * (BW // P) + j2, :], ps)

            # ---------------- phase 2 + 3: attention + out-proj ----------
            with tc.tile_pool(name="expp", bufs=6) as expp, \
                 tc.tile_pool(name="treep", bufs=6) as treep, \
                 tc.tile_pool(name="rcp", bufs=2) as rcp, \
                 tc.tile_pool(name="ysp", bufs=4) as ysp, \
                 tc.tile_pool(name="dotsp", bufs=2, space="PSUM") as dotsp, \
                 tc.tile_pool(name="avp", bufs=1, space="PSUM") as avp, \
                 tc.tile_pool(name="psyp", bufs=1, space="PSUM") as psyp:

                iters = [(ib, h) for ib in range(IB2) for h in range(HH)]
                pv_pend = deque()      # (k, jd, et)
                ph3_queue = deque()    # (ib, e8, hf)
                state = {}             # k -> dict(acc, pd, rc, po)

                def feed_tree(levels, cur):
                    lvl = 0
                    while levels[lvl] is not None:
                        prev = levels[lvl]
                        levels[lvl] = None
                        with nc.allow_low_precision("softmax denom tree bf16"):
                            dst = treep.tile([P, 1024], bf16, tag="tree",
                                             name="tree")
                            nc.vector.tensor_add(dst, prev, cur)
                        cur = dst
                        lvl += 1
                    levels[lvl] = cur

                def emit_pv(kk, jd, et):
                    st = state[kk]
                    _, hh_ = iters[kk]
                    v_l = v_sb[:, jd, hh_ * P:(hh_ + 1) * P]
                    for hf in range(2):
                        sl = slice(hf * 512, (hf + 1) * 512)
                        nc.tensor.matmul(
                            st["po"][:, sl], v_l, et[:, sl],
                            start=(jd == 0), stop=(jd == NJT - 1))
                    if jd == NJT - 1:
                        # drain PV accumulator right away so the single
                        # avp buffer frees for the next iteration
                        ib_, hh2 = iters[kk]
                        nc.vector.tensor_copy(o_ts[hh2][ib_], st["po"])

                def emit_den(kk):
                    st = state[kk]
                    pd = psyp.tile([P, 1024], f32, tag="psy", name="pden")
                    for hf in range(2):
                        sl = slice(hf * 512, (hf + 1) * 512)
                        nc.tensor.matmul(pd[:, sl], ones_sb, st["acc"][:, sl],
                                         start=True, stop=True)
                    st["pd"] = pd

                def emit_recip(kk):
                    st = state[kk]
                    rc = rcp.tile([P, 1024], f32, tag="rc", name="rc")
                    nc.vector.reciprocal_approx_fast(rc, st["pd"])
                    st["rc"] = rc

                def emit_norm(kk):
                    st = state[kk]
                    ib_, hh_ = iters[kk]
                    osl = o_ts[hh_][ib_]
                    with nc.allow_low_precision("softmax normalize bf16"):
                        nc.vector.tensor_mul(osl, osl, st["rc"])

                def emit_ph3_chunk(ib_, e8, hf, copy_eng="v"):
                    pt = psyp.tile([P, 1024], f32, tag="psy", name="py")
                    pz = pt[:, 0:512]
                    for dt in range(HH):
                        nc.tensor.matmul(
                            pz, wo_sb[:, dt, e8 * P:(e8 + 1) * P],
                            o_ts[dt][ib_][:, hf * 512:(hf + 1) * 512],
                            start=(dt == 0), stop=(dt == HH - 1))
                    ys = ysp.tile([P, 512], bf16, tag="ys", name="ys")
                    if copy_eng == "v":
                        nc.vector.tensor_copy(ys, pz)
                    else:
                        nc.scalar.copy(ys, pz)
                    i0 = ib_ * 1024 + hf * 512
                    nc.sync.dma_start(
                        out=yT[e8 * P:(e8 + 1) * P, i0:i0 + 512], in_=ys)

                for k, (ib, h) in enumerate(iters):
                    st = {"po": avp.tile([P, 1024], f32, tag="po", name="po")}
                    state[k] = st
                    levels = [None] * 5
                    i_base = ib * 1024

                    for jt in range(NJT):
                        # deferred post-iteration work for k-1, spread over
                        # early steps so nothing serializes the PE
                        if k > 0:
                            if jt == 2:
                                emit_den(k - 1)
                            elif jt == 3:
                                emit_recip(k - 1)
                            elif jt == 5:
                                emit_norm(k - 1)

                        # dots
                        ps = dotsp.tile([P, 1024], f32, tag="ps", name="ps")
                        k_l = kT_sb[:, h, jt * P:(jt + 1) * P]
                        for hf in range(2):
                            nc.tensor.matmul(
                                ps[:, hf * 512:(hf + 1) * 512], k_l,
                                qT_sb[:, h,
                                      i_base + hf * 512:i_base + (hf + 1) * 512],
                                start=True, stop=True)
                        et = expp.tile([P, 1024], bf16, tag="exp", name="et")
                        nc.scalar.activation(et, ps, Exp, scale=SCALE)
                        feed_tree(levels, et)
                        pv_pend.append((k, jt, et))
                        if len(pv_pend) > LAG:
                            emit_pv(*pv_pend.popleft())

                        # interleave phase-3 chunks of the previous i-block
                        # into the PE slack (ACT-bound steps); jt>=5 keeps
                        # the psy slot free for den/recip of k-1 and (at
                        # k==4) waits for osl(ib0,h3) to be normalized
                        if ph3_queue and jt in (5, 8, 11, 14):
                            emit_ph3_chunk(*ph3_queue.popleft())

                    st["acc"] = levels[4]
                    assert st["acc"] is not None

                    if h == HH - 1:
                        for e8 in range(DIM // P):
                            for hf in range(2):
                                ph3_queue.append((ib, e8, hf))

                # tail: drain the pipeline for the last iteration.  The ib1
                # out-proj chunks run 2-deep through the (now free) dots
                # PSUM pool, with their dt<3 accumulations pre-running while
                # the final denominator chain completes on DVE, so the PE
                # never idles long enough for HAM to re-throttle the clock.
                while pv_pend:
                    emit_pv(*pv_pend.popleft())
                kl = len(iters) - 1
                tail = list(ph3_queue)
                ph3_queue.clear()
                open_ps = {}
                LOOK = 3   # 3 open out-proj groups: 2 dotsp bufs + the avp
                           # buffer (free once po[kl] is drained)

                def tail_stage_a(i):
                    ib_, e8, hf = tail[i]
                    if i % 3 == 2:
                        pt = avp.tile([P, 1024], f32, tag="po", name="po_t")
                    else:
                        pt = dotsp.tile([P, 1024], f32, tag="ps", name="py2")
                    pz = pt[:, 0:512]
                    for dt in range(HH - 1):
                        nc.tensor.matmul(
                            pz, wo_sb[:, dt, e8 * P:(e8 + 1) * P],
                            o_ts[dt][ib_][:, hf * 512:(hf + 1) * 512],
                            start=(dt == 0), stop=False)
                    open_ps[i] = pt

                def tail_stage_b(j):
                    ib_, e8, hf = tail[j]
                    pt = open_ps.pop(j)
                    pz = pt[:, 0:512]
                    nc.tensor.matmul(
                        pz, wo_sb[:, HH - 1, e8 * P:(e8 + 1) * P],
                        o_ts[HH - 1][ib_][:, hf * 512:(hf + 1) * 512],
                        start=False, stop=True)
                    # first copies go to ACT (DVE still owes the final
                    # reciprocal/normalize chain), then alternate engines
                    ys = ysp.tile([P, 512], bf16, tag="ys", name="ys")
                    if j < 4 or j % 2 == 0:
                        nc.scalar.copy(ys, pz)
                    else:
                        nc.vector.tensor_copy(ys, pz)
                    i0 = ib_ * 1024 + hf * 512
                    nc.sync.dma_start(
                        out=yT[e8 * P:(e8 + 1) * P, i0:i0 + 512], in_=ys)

                for i in range(len(tail) + LOOK):
                    if i >= LOOK:
                        tail_stage_b(i - LOOK)
                    if i < len(tail):
                        tail_stage_a(i)
                    if i == 0:
                        emit_den(kl)
                        emit_recip(kl)
                    elif i == 1:
                        emit_norm(kl)

    nc.compile()
    return nc


_nc_by_reps = {}


def _get_nc(reps=1):
    if reps not in _nc_by_reps:
        _nc_by_reps[reps] = _build_nc(reps)
    return _nc_by_reps[reps]


def _make_in_maps(x_a, x_b, W_q, W_kv, W_out):
    import ml_dtypes
    bf = ml_dtypes.bfloat16
    xaT = [np.ascontiguousarray(x_a[b].T).astype(bf) for b in range(B)]
    xbT = [np.ascontiguousarray(x_b[b].T).astype(bf) for b in range(B)]
    ones = np.ones((P, P), dtype=bf)
    in_maps = []
    for c in range(NCORES):
        b, hh = divmod(c, 2)
        hs = hh * DVC
        in_maps.append({
            "xaT": xaT[b],
            "xbT": xbT[b],
            "wqT": np.ascontiguousarray(W_q[hs:hs + DVC].T).astype(bf),
            "wkT": np.ascontiguousarray(W_kv[hs:hs + DVC].T).astype(bf),
            "wvT": np.ascontiguousarray(
                W_kv[DIM + hs:DIM + hs + DVC].T).astype(bf),
            "woT": np.ascontiguousarray(W_out[:, hs:hs + DVC].T).astype(bf),
            "ones": ones,
        })
    return in_maps


def kernel(x_a, x_b, W_q, W_kv, W_out, b_out):
    global LAST_EXEC_NS, LAST_RESULTS
    from concourse import bass_utils

    x_a = np.asarray(x_a, dtype=np.float32)
    x_b = np.asarray(x_b, dtype=np.float32)
    W_q = np.asarray(W_q, dtype=np.float32)
    W_kv = np.asarray(W_kv, dtype=np.float32)
    W_out = np.asarray(W_out, dtype=np.float32)
    b_out = np.asarray(b_out, dtype=np.float32)

    nc = _get_nc(REPS)
    in_maps = _make_in_maps(x_a, x_b, W_q, W_kv, W_out)

    res = bass_utils.run_bass_kernel_spmd(
        nc, in_maps, core_ids=list(range(NCORES)), trace=_TRACE,
        tmpdir=_TRACE_DIR)
    LAST_EXEC_NS = res.exec_time_ns
    LAST_RESULTS = res

    out = np.empty((B, N, DIM), dtype=np.float32)
    for b in range(B):
        acc = (np.asarray(res.results[2 * b]["yT"]).astype(np.float32)
               + np.asarray(res.results[2 * b + 1]["yT"]).astype(np.float32))
        out[b] = acc.T + b_out
    return out


def bench(inputs, reps_pair=(1, 9), iters=5):
    """Measure on-device time per kernel body via rep-delta wall timing."""
    import time
    from concourse import bass_utils
    ins = {k: np.asarray(v, dtype=np.float32) for k, v in inputs.items()
           if k != "b_out"}
    in_maps = _make_in_maps(ins["x_a"], ins["x_b"], ins["W_q"], ins["W_kv"],
                            ins["W_out"])
    walls = {}
    for reps in reps_pair:
        nc = _get_nc(reps)
        bass_utils.run_bass_kernel_spmd(nc, in_maps, core_ids=list(range(NCORES)))
        ts = []
        for _ in range(iters):
            t0 = time.perf_counter()
            bass_utils.run_bass_kernel_spmd(nc, in_maps,
                                            core_ids=list(range(NCORES)))
            ts.append(time.perf_counter() - t0)
        walls[reps] = min(ts)
        print(f"reps={reps}: wall min={walls[reps]*1e3:.2f} ms  "
              f"all={[f'{t*1e3:.1f}' for t in ts]}")
    r0, r1 = reps_pair
    ns = (walls[r1] - walls[r0]) / (r1 - r0) * 1e9
    print(f"per-body device time: {ns:.0f} ns")
    return ns


# revision 17
# speedup vs baseline: 1.0055x; 1.0009x over previous
"""Cross-modal attention TRN2 kernel (v2, bf16).

Problem: B=4, N=2048, IN_DIM=DIM=1024, HEADS=8, D_HEAD=128, scale=DIM**-0.5.
  q = x_a @ W_q.T ; k,v = split(x_b @ W_kv.T) ; per-head softmax(q k^T/32) v ;
  out = merge_heads @ W_out.T + b_out

Sharding over 8 cores: core c -> batch b=c//2, head-half hh=c%2 (4 heads,
512 of DIM).  W_q/W_kv column-sharded, W_out row-sharded (Megatron); each
core emits a partial output projection y_cT = (W_out[:, slice] @ O_half)
of shape [DIM, N] in bf16; host sums the two head-half partials per batch
in fp32, adds b_out, transposes back.

v2 changes vs v1 (fp32r, 401us):
  - all matmul operands bf16 (same 1 cyc/row PE rate, but half DMA, half
    LDWEIGHTS, FWL enabled, 2x DVE rates).  PSUM accumulation stays fp32.
  - softmax denominator no longer computed with per-j-tile ones-matmuls
    (which cost as much PE time as the PV matmuls).  Instead the exp tiles
    are summed over j-tiles with a bf16 binary add-tree on the Vector
    engine, and one [128,128] all-ones stationary matmul per (ib,h) both
    reduces over the 128 j-partitions and broadcasts the denominator to
    all 128 partitions of a PSUM tile.  reciprocal_approx_fast (DVE custom
    op, ~5x faster than InstReciprocal) gives 1/den at [128,1024] shape,
    so nothing runs at 1-partition serial rates and the per-iteration
    normalize chain is off the PE critical path.
  - phase 2 software-pipelined ACROSS (ib,h) iterations (PE never drains,
    so the HAM clock gate stays at 2.4 GHz), with the phase-3 output
    projection matmuls for i-block 0 interleaved into the PE slack of the
    ACT-bound (exp-bound) attention iterations of i-block 1.
"""

import numpy as np
from collections import deque

B, N, IN_DIM, DIM, HEADS = 4, 2048, 1024, 1024, 8
D_HEAD = DIM // HEADS          # 128
SCALE = DIM ** -0.5            # 1/32
NCORES = 8
HH = HEADS // 2                # 4 heads per core
DVC = HH * D_HEAD              # 512 dv per core
P = 128
KT = IN_DIM // P               # 8 contraction tiles
NJT = N // P                   # 16 j tiles
IB2 = N // 1024                # 2 i-blocks of 1024
LAG = 2                        # PV trails dots/exp by LAG j-tiles

_TRACE = False
_TRACE_DIR = None
REPS = 1
LAST_EXEC_NS = None
LAST_RESULTS = None


def _build_nc(reps=1):
    import concourse.tile as tile
    from concourse import bacc, mybir

    f32 = mybir.dt.float32
    bf16 = mybir.dt.bfloat16
    Exp = mybir.ActivationFunctionType.Exp

    nc = bacc.Bacc("TRN2", debug=False, num_devices=NCORES)

    # Everything bf16 (fp8e4 DoubleRow q/k projections were measured at
    # rel_err 1.96e-2 vs the 2e-2 gate -- too thin; bf16 runs at 6.5e-3).
    xaT = nc.dram_tensor("xaT", [IN_DIM, N], bf16, kind="ExternalInput").ap()
    xbT = nc.dram_tensor("xbT", [IN_DIM, N], bf16, kind="ExternalInput").ap()
    wqT = nc.dram_tensor("wqT", [IN_DIM, DVC], bf16, kind="ExternalInput").ap()
    wkT = nc.dram_tensor("wkT", [IN_DIM, DVC], bf16, kind="ExternalInput").ap()
    wvT = nc.dram_tensor("wvT", [IN_DIM, DVC], bf16, kind="ExternalInput").ap()
    woT = nc.dram_tensor("woT", [DVC, DIM], bf16, kind="ExternalInput").ap()
    ones_d = nc.dram_tensor("ones", [P, P], bf16, kind="ExternalInput").ap()
    yT = nc.dram_tensor("yT", [DIM, N], bf16, kind="ExternalOutput").ap()

    with tile.TileContext(nc) as tc:
      for _rep in range(reps):
        with tc.tile_pool(name="persist", bufs=1) as persist:
            qT_sb = persist.tile([P, HH, N], bf16, tag="qT")    # [d%128, h, i]
            kT_sb = persist.tile([P, HH, N], bf16, tag="kT")    # [d%128, h, j]
            v_sb = persist.tile([P, NJT, DVC], bf16, tag="v")   # [j%128, jt, dv]
            o_ts = [[persist.tile([P, 1024], bf16, tag=f"o{h}_{bb}",
                                  name=f"o{h}_{bb}")
                     for bb in range(IB2)] for h in range(HH)]
            ones_sb = persist.tile([P, P], bf16, tag="ones")
            wo_sb = persist.tile([P, HH, DIM], bf16, tag="wo")  # [dv%128, h, e]

            # ---------------- phase 1: projections ----------------
            BW = 512
            NB = N // BW                                        # 4 blocks
            with tc.tile_pool(name="wpool", bufs=1) as wpool, \
                 tc.tile_pool(name="xapool", bufs=2) as xapool, \
                 tc.tile_pool(name="xbpool", bufs=2) as xbpool, \
                 tc.tile_pool(name="psum1", bufs=4, space="PSUM") as psum1:
                wq_sb = wpool.tile([P, KT, DVC], bf16, tag="wq")
                wk_sb = wpool.tile([P, KT, DVC], bf16, tag="wk")
                wv_sb = wpool.tile([P, KT, DVC], bf16, tag="wv")

                def new_xa(blk, halves=1):
                    t = xapool.tile([P, KT, BW], bf16, tag="xa", name="xa_blk")
                    hk = KT // halves
                    for hv in range(halves):
                        nc.sync.dma_start(
                            out=t[:, hv * hk:(hv + 1) * hk, :],
                            in_=xaT[hv * hk * P:(hv + 1) * hk * P,
                                    blk * BW:(blk + 1) * BW]
                            .rearrange("(kt p) i -> p kt i", p=P))
                    return t

                def new_xb(blk, halves=1):
                    t = xbpool.tile([P, KT, BW], bf16, tag="xb", name="xb_blk")
                    hk = KT // halves
                    for hv in range(halves):
                        nc.sync.dma_start(
                            out=t[:, hv * hk:(hv + 1) * hk, :],
                            in_=xbT[hv * hk * P:(hv + 1) * hk * P,
                                    blk * BW:(blk + 1) * BW]
                            .rearrange("(kt p) i -> p kt i", p=P))
                    return t

                # DMA order: earliest-needed first; block 0 and the weights
                # are split in kt-halves interleaved so the first Q matmul
                # can issue ~3us in instead of waiting out both full loads.
                def dma_w_half(dst, src, hv):
                    nc.sync.dma_start(
                        out=dst[:, hv * 4:(hv + 1) * 4, :],
                        in_=src[hv * 4 * P:(hv + 1) * 4 * P, :]
                        .rearrange("(kt p) d -> p kt d", p=P))

                xa_blk = xapool.tile([P, KT, BW], bf16, tag="xa",
                                     name="xa_blk")
                nc.sync.dma_start(
                    out=xa_blk[:, 0:4, :],
                    in_=xaT[0:4 * P, 0:BW].rearrange("(kt p) i -> p kt i",
                                                     p=P))
                dma_w_half(wq_sb, wqT, 0)
                nc.sync.dma_start(
                    out=xa_blk[:, 4:8, :],
                    in_=xaT[4 * P:8 * P, 0:BW].rearrange("(kt p) i -> p kt i",
                                                         p=P))
                dma_w_half(wq_sb, wqT, 1)
                xb_blk = new_xb(0, halves=2)
                dma_w_half(wk_sb, wkT, 0)
                dma_w_half(wk_sb, wkT, 1)
                nc.sync.dma_start(
                    out=wv_sb, in_=wvT.rearrange("(kt p) d -> p kt d", p=P))
                nc.sync.dma_start(out=ones_sb, in_=ones_d)
                nc.sync.dma_start(
                    out=wo_sb, in_=woT.rearrange("(dt p) e -> p dt e", p=P))

                for blk in range(NB):
                    if blk > 0:
                        xa_blk = new_xa(blk)
                        xb_blk = new_xb(blk)
                    # Q block
                    for dt in range(HH):
                        ps = psum1.tile([P, BW], f32, tag="ps1", name="ps1")
                        for kt in range(KT):
                            nc.tensor.matmul(
                                ps, wq_sb[:, kt, dt * P:(dt + 1) * P],
                                xa_blk[:, kt, :],
                                start=(kt == 0), stop=(kt == KT - 1))
                        nc.vector.tensor_copy(
                            qT_sb[:, dt, blk * BW:(blk + 1) * BW], ps)
                    # K block
                    for dt in range(HH):
                        ps = psum1.tile([P, BW], f32, tag="ps1", name="ps1")
                        for kt in range(KT):
                            nc.tensor.matmul(
                                ps, wk_sb[:, kt, dt * P:(dt + 1) * P],
                                xb_blk[:, kt, :],
                                start=(kt == 0), stop=(kt == KT - 1))
                        nc.vector.tensor_copy(
                            kT_sb[:, dt, blk * BW:(blk + 1) * BW], ps)
                    # V block (j-partitioned: stationary = x slice)
                    for j2 in range(BW // P):
                        ps = psum1.tile([P, DVC], f32, tag="ps1", name="psv")
                        for kt in range(KT):
                            nc.tensor.matmul(
                                ps, xb_blk[:, kt, j2 * P:(j2 + 1) * P],
                                wv_sb[:, kt, :],
                                start=(kt == 0), stop=(kt == KT - 1))
                        nc.vector.tensor_copy(
                            v_sb[:, blk * (BW // P) + j2, :], ps)

            # ---------------- phase 2 + 3: attention + out-proj ----------
            with tc.tile_pool(name="expp", bufs=6) as expp, \
                 tc.tile_pool(name="treep", bufs=6) as treep, \
                 tc.tile_pool(name="rcp", bufs=2) as rcp, \
                 tc.tile_pool(name="ysp", bufs=4) as ysp, \
                 tc.tile_pool(name="dotsp", bufs=2, space="PSUM") as dotsp, \
                 tc.tile_pool(name="avp", bufs=1, space="PSUM") as avp, \
                 tc.tile_pool(name="psyp", bufs=1, space="PSUM") as psyp:

                iters = [(ib, h) for ib in range(IB2) for h in range(HH)]
                pv_pend = deque()      # (k, jd, et)
                ph3_queue = deque()    # (ib, e8, hf)
                state = {}             # k -> dict(acc, pd, rc, po)

                def feed_tree(levels, cur):
                    lvl = 0
                    while levels[lvl] is not None:
                        prev = levels[lvl]
                        levels[lvl] = None
                        with nc.allow_low_precision("softmax denom tree bf16"):
                            dst = treep.tile([P, 1024], bf16, tag="tree",
                                             name="tree")
                            nc.vector.tensor_add(dst, prev, cur)
                        cur = dst
                        lvl += 1
                    levels[lvl] = cur

                def emit_pv(kk, jd, et):
                    st = state[kk]
                    _, hh_ = iters[kk]
                    v_l = v_sb[:, jd, hh_ * P:(hh_ + 1) * P]
                    for hf in range(2):
                        sl = slice(hf * 512, (hf + 1) * 512)
                        nc.tensor.matmul(
                            st["po"][:, sl], v_l, et[:, sl],
                            start=(jd == 0), stop=(jd == NJT - 1))
                    if jd == NJT - 1:
                        # drain PV accumulator right away so the single
                        # avp buffer frees for the next iteration
                        ib_, hh2 = iters[kk]
                        nc.vector.tensor_copy(o_ts[hh2][ib_], st["po"])

                def emit_den(kk):
                    st = state[kk]
                    pd = psyp.tile([P, 1024], f32, tag="psy", name="pden")
                    for hf in range(2):
                        sl = slice(hf * 512, (hf + 1) * 512)
                        nc.tensor.matmul(pd[:, sl], ones_sb, st["acc"][:, sl],
                                         start=True, stop=True)
                    st["pd"] = pd

                def emit_recip(kk):
                    st = state[kk]
                    rc = rcp.tile([P, 1024], f32, tag="rc", name="rc")
                    nc.vector.reciprocal_approx_fast(rc, st["pd"])
                    st["rc"] = rc

                def emit_norm(kk):
                    st = state[kk]
                    ib_, hh_ = iters[kk]
                    osl = o_ts[hh_][ib_]
                    with nc.allow_low_precision("softmax normalize bf16"):
                        nc.vector.tensor_mul(osl, osl, st["rc"])

                def emit_ph3_chunk(ib_, e8, hf, copy_eng="v"):
                    pt = psyp.tile([P, 1024], f32, tag="psy", name="py")
                    pz = pt[:, 0:512]
                    for dt in range(HH):
                        nc.tensor.matmul(
                            pz, wo_sb[:, dt, e8 * P:(e8 + 1) * P],
                            o_ts[dt][ib_][:, hf * 512:(hf + 1) * 512],
                            start=(dt == 0), stop=(dt == HH - 1))
                    ys = ysp.tile([P, 512], bf16, tag="ys", name="ys")
                    if copy_eng == "v":
                        nc.vector.tensor_copy(ys, pz)
                    else:
                        nc.scalar.copy(ys, pz)
                    i0 = ib_ * 1024 + hf * 512
                    nc.sync.dma_start(
                        out=yT[e8 * P:(e8 + 1) * P, i0:i0 + 512], in_=ys)

                for k, (ib, h) in enumerate(iters):
                    st = {"po": avp.tile([P, 1024], f32, tag="po", name="po")}
                    state[k] = st
                    levels = [None] * 5
                    i_base = ib * 1024

                    for jt in range(NJT):
                        # deferred post-iteration work for k-1, spread over
                        # early steps so nothing serializes the PE
                        if k > 0:
                            if jt == 2:
                                emit_den(k - 1)
                            elif jt == 3:
                                emit_recip(k - 1)
                            elif jt == 5:
                                emit_norm(k - 1)

                        # dots
                        ps = dotsp.tile([P, 1024], f32, tag="ps", name="ps")
                        k_l = kT_sb[:, h, jt * P:(jt + 1) * P]
                        for hf in range(2):
                            nc.tensor.matmul(
                                ps[:, hf * 512:(hf + 1) * 512], k_l,
                                qT_sb[:, h,
                                      i_base + hf * 512:i_base + (hf + 1) * 512],
                                start=True, stop=True)
                        et = expp.tile([P, 1024], bf16, tag="exp", name="et")
                        nc.scalar.activation(et, ps, Exp, scale=SCALE)
                        feed_tree(levels, et)
                        pv_pend.append((k, jt, et))
                        if len(pv_pend) > LAG:
                            emit_pv(*pv_pend.popleft())

                        # interleave phase-3 chunks of the previous i-block
                        # into the PE slack (ACT-bound steps); jt>=5 keeps
                        # the psy slot free for den/recip of k-1 and (at
                        # k==4) waits for osl(ib0,h3) to be normalized
                        if ph3_queue and jt in (5, 8, 11, 14):
                            emit_ph3_chunk(*ph3_queue.popleft())

                    st["acc"] = levels[4]
                    assert st["acc"] is not None

                    if h == HH - 1:
                        for e8 in range(DIM // P):
                            for hf in range(2):
                                ph3_queue.append((ib, e8, hf))

                # tail: drain the pipeline for the last iteration.  The ib1
                # out-proj chunks run 2-deep through the (now free) dots
                # PSUM pool, with their dt<3 accumulations pre-running while
                # the final denominator chain completes on DVE, so the PE
                # never idles long enough for HAM to re-throttle the clock.
                while pv_pend:
                    emit_pv(*pv_pend.popleft())
                kl = len(iters) - 1
                tail = list(ph3_queue)
                ph3_queue.clear()
                open_ps = {}
                LOOK = 3   # 3 open out-proj groups: 2 dotsp bufs + the avp
                           # buffer (free once po[kl] is drained)

                def tail_stage_a(i):
                    ib_, e8, hf = tail[i]
                    if i % 3 == 2:
                        pt = avp.tile([P, 1024], f32, tag="po", name="po_t")
                    else:
                        pt = dotsp.tile([P, 1024], f32, tag="ps", name="py2")
                    pz = pt[:, 0:512]
                    for dt in range(HH - 1):
                        nc.tensor.matmul(
                            pz, wo_sb[:, dt, e8 * P:(e8 + 1) * P],
                            o_ts[dt][ib_][:, hf * 512:(hf + 1) * 512],
                            start=(dt == 0), stop=False)
                    open_ps[i] = pt

                def tail_stage_b(j):
                    ib_, e8, hf = tail[j]
                    pt = open_ps.pop(j)
                    pz = pt[:, 0:512]
                    nc.tensor.matmul(
                        pz, wo_sb[:, HH - 1, e8 * P:(e8 + 1) * P],
                        o_ts[HH - 1][ib_][:, hf * 512:(hf + 1) * 512],
                        start=False, stop=True)
                    # first copies go to ACT (DVE still owes the final
                    # reciprocal/normalize chain), then alternate engines
                    ys = ysp.tile([P, 512], bf16, tag="ys", name="ys")
                    if j < 4 or j % 2 == 0:
                        nc.scalar.copy(ys, pz)
                    else:
                        nc.vector.tensor_copy(ys, pz)
                    i0 = ib_ * 1024 + hf * 512
                    nc.sync.dma_start(
                        out=yT[e8 * P:(e8 + 1) * P, i0:i0 + 512], in_=ys)

                # den/recip go first so the DVE normalize chain (which
                # gates every chunk's dt3 closure) starts as early as
                # possible; the PV drain above keeps the PE covered while
                # the add-tree finishes.
                emit_den(kl)
                emit_recip(kl)
                for i in range(len(tail) + LOOK):
                    if i >= LOOK:
                        tail_stage_b(i - LOOK)
                    if i < len(tail):
                        tail_stage_a(i)
                    if i == 0:
                        emit_norm(kl)

    nc.compile()
    return nc


_nc_by_reps = {}


def _get_nc(reps=1):
    if reps not in _nc_by_reps:
        _nc_by_reps[reps] = _build_nc(reps)
    return _nc_by_reps[reps]


def _make_in_maps(x_a, x_b, W_q, W_kv, W_out):
    import ml_dtypes
    bf = ml_dtypes.bfloat16
    xaT = [np.ascontiguousarray(x_a[b].T).astype(bf) for b in range(B)]
    xbT = [np.ascontiguousarray(x_b[b].T).astype(bf) for b in range(B)]
    ones = np.ones((P, P), dtype=bf)
    in_maps = []
    for c in range(NCORES):
        b, hh = divmod(c, 2)
        hs = hh * DVC
        in_maps.append({
            "xaT": xaT[b],
            "xbT": xbT[b],
            "wqT": np.ascontiguousarray(W_q[hs:hs + DVC].T).astype(bf),
            "wkT": np.ascontiguousarray(W_kv[hs:hs + DVC].T).astype(bf),
            "wvT": np.ascontiguousarray(
                W_kv[DIM + hs:DIM + hs + DVC].T).astype(bf),
            "woT": np.ascontiguousarray(W_out[:, hs:hs + DVC].T).astype(bf),
            "ones": ones,
        })
    return in_maps


def kernel(x_a, x_b, W_q, W_kv, W_out, b_out):
    global LAST_EXEC_NS, LAST_RESULTS
    from concourse import bass_utils

    x_a = np.asarray(x_a, dtype=np.float32)
    x_b = np.asarray(x_b, dtype=np.float32)
    W_q = np.asarray(W_q, dtype=np.float32)
    W_kv = np.asarray(W_kv, dtype=np.float32)
    W_out = np.asarray(W_out, dtype=np.float32)
    b_out = np.asarray(b_out, dtype=np.float32)

    nc = _get_nc(REPS)
    in_maps = _make_in_maps(x_a, x_b, W_q, W_kv, W_out)

    res = bass_utils.run_bass_kernel_spmd(
        nc, in_maps, core_ids=list(range(NCORES)), trace=_TRACE,
        tmpdir=_TRACE_DIR)
    LAST_EXEC_NS = res.exec_time_ns
    LAST_RESULTS = res

    out = np.empty((B, N, DIM), dtype=np.float32)
    for b in range(B):
        acc = (np.asarray(res.results[2 * b]["yT"]).astype(np.float32)
               + np.asarray(res.results[2 * b + 1]["yT"]).astype(np.float32))
        out[b] = acc.T + b_out
    return out


def bench(inputs, reps_pair=(1, 9), iters=5):
    """Measure on-device time per kernel body via rep-delta wall timing."""
    import time
    from concourse import bass_utils
    ins = {k: np.asarray(v, dtype=np.float32) for k, v in inputs.items()
           if k != "b_out"}
    in_maps = _make_in_maps(ins["x_a"], ins["x_b"], ins["W_q"], ins["W_kv"],
                            ins["W_out"])
    walls = {}
    for reps in reps_pair:
        nc = _get_nc(reps)
        bass_utils.run_bass_kernel_spmd(nc, in_maps, core_ids=list(range(NCORES)))
        ts = []
        for _ in range(iters):
            t0 = time.perf_counter()
            bass_utils.run_bass_kernel_spmd(nc, in_maps,
                                            core_ids=list(range(NCORES)))
            ts.append(time.perf_counter() - t0)
        walls[reps] = min(ts)
        print(f"reps={reps}: wall min={walls[reps]*1e3:.2f} ms  "
              f"all={[f'{t*1e3:.1f}' for t in ts]}")
    r0, r1 = reps_pair
    ns = (walls[r1] - walls[r0]) / (r1 - r0) * 1e9
    print(f"per-body device time: {ns:.0f} ns")
    return ns
